# revision 1
# baseline (speedup 1.0000x reference)
"""Bayesian linear layer (per-sample weights) on 8 Trainium2 NeuronCores.

out[b,o] = sum_i x[b,i] * (eps[b,i,o]*softplus(ro)[i,o] + mu[i,o])
           + eps_bias[b,o]*softplus(ro_bias)[o] + mu_bias[o]

Strategy (2D sharding: 4 batch-groups x 2 i-halves per core):
  - Each core handles 32 samples and 512 of the 1024 contraction rows,
    producing a partial sum; the host unshard adds the two i-halves.
    This halves the replicated ro/mu traffic (HBM-domain bandwidth,
    shared by NC pairs, is the binding resource) while keeping every
    per-partition DMA run 16KB contiguous: contraction rows are mapped
    p-major (i_local = 4p + c), which the host mirrors in the x layout.
  - eps rows for one sample ([512, 1024] f32, 2MB contiguous) stream as
    one [128, 4096] tile on the sync HWDGE ring, which carries nothing
    else; params ride the scalar ring, misc the gpsimd ring.
  - DVE multiplies tiles by softplus(ro), rounding to float32r so
    TensorE consumes them at full (1 cycle/row) rate.
  - TensorE reduces over i with M=1 matmuls (lhsT = x column) into a
    [1,1024] PSUM tile per sample; a one-hot K=32 matmul folds in the
    bias row (x@mu_half + bias terms on the j=0 core; zeros on j=1),
    the scalar engine copies PSUM -> SBUF and stores via its ring.
"""

import numpy as np

import concourse.bass as bass
import concourse.bacc as bacc
import concourse.mybir as mybir
from concourse.masks import make_identity
from concourse.tile import TileContext
from concourse.bass_utils import run_bass_kernel_spmd

F32 = mybir.dt.float32
F32R = mybir.dt.float32r
AF = mybir.ActivationFunctionType

B, IN, OUT = 128, 1024, 1024
NCORES = 8
BG = 4                    # batch groups
ISH = NCORES // BG        # i-shards (2)
BS = B // BG              # 32 samples per core
INS = IN // ISH           # 512 contraction rows per core
P = 128
CPP = INS // P            # 4 contraction rows per partition
FREE = CPP * OUT          # 4096 free elems per eps tile (one sample)


def build_nc():
    nc = bacc.Bacc(None, target_bir_lowering=False)

    eps_d = nc.declare_dram_parameter("eps", [BS, INS, OUT], F32, isOutput=False)
    ro_d = nc.declare_dram_parameter("ro", [INS, OUT], F32, isOutput=False)
    mu_d = nc.declare_dram_parameter("mu", [INS, OUT], F32, isOutput=False)
    # xt[p, c*BS + b] = x[b, ishard*512 + c*128 + p]  (host-side layout)
    xt_d = nc.declare_dram_parameter("xt", [P, CPP * BS], F32, isOutput=False)
    eb_d = nc.declare_dram_parameter("eps_bias", [BS, OUT], F32, isOutput=False)
    rb_d = nc.declare_dram_parameter("ro_bias", [BS, OUT], F32, isOutput=False)
    mb_d = nc.declare_dram_parameter("mu_bias", [BS, OUT], F32, isOutput=False)
    out_d = nc.declare_dram_parameter("out", [BS, OUT], F32, isOutput=True)

    # i_local = c*128 + p: chunk-major, 4KB per-partition DMA runs
    ro_r = ro_d.rearrange("(c p) o -> p c o", p=P)
    mu_r = mu_d.rearrange("(c p) o -> p c o", p=P)

    with TileContext(nc) as tc:
        with (
            tc.tile_pool(name="const", bufs=1) as cpool,
            tc.tile_pool(name="eps", bufs=5) as epool,
            tc.tile_pool(name="epr", bufs=3) as eprpool,
            tc.tile_pool(name="small", bufs=2) as spool,
            tc.tile_pool(name="psmu", bufs=1, space="PSUM") as pmupool,
            tc.tile_pool(name="psum", bufs=3, space="PSUM") as ppool,
        ):
            # ---- softplus(ro): quarters lead the single (sync) DMA ring -
            sig = cpool.tile([P, FREE], F32)
            for h in range(CPP):
                sl = sig[:, h * OUT : (h + 1) * OUT]
                nc.sync.dma_start(out=sl, in_=ro_r[:, h : h + 1, :])
                nc.scalar.activation(sl, sl, AF.Exp)
                nc.scalar.activation(sl, sl, AF.Ln, bias=1.0)

            xt = cpool.tile([P, CPP * BS], F32)
            nc.sync.dma_start(out=xt, in_=xt_d[:, :])
            xtr = cpool.tile([P, CPP * BS], F32R)
            nc.vector.tensor_copy(out=xtr, in_=xt)

            ident = cpool.tile([BS, BS], F32)
            make_identity(nc, ident)
            idr = cpool.tile([BS, BS], F32R)
            nc.vector.tensor_copy(out=idr, in_=ident)

            # ---- x @ mu (partial over this core's i rows) ---------------
            psmu = pmupool.tile([BS, OUT], F32)
            mt = epool.tile([P, FREE], F32, tag="ep")
            nc.sync.dma_start(out=mt, in_=mu_r[:, :, :])
            for c in range(CPP):
                for nh in range(2):
                    nc.tensor.matmul(
                        psmu[:, nh * 512 : (nh + 1) * 512],
                        xt[:, c * BS : (c + 1) * BS],
                        mt[:, c * OUT + nh * 512 : c * OUT + (nh + 1) * 512],
                        start=(c == 0),
                        stop=(c == CPP - 1),
                    )

            # ---- bias row (j=0 core: real biases; j=1 core: zeros) ------
            eb16 = cpool.tile([BS, OUT], F32)
            nc.sync.dma_start(out=eb16, in_=eb_d[:, :])
            rb16 = cpool.tile([BS, OUT], F32)
            nc.sync.dma_start(out=rb16, in_=rb_d[:, :])
            mb16 = cpool.tile([BS, OUT], F32)
            nc.sync.dma_start(out=mb16, in_=mb_d[:, :])
            nc.scalar.activation(rb16, rb16, AF.Exp)
            nc.scalar.activation(rb16, rb16, AF.Ln, bias=1.0)

            nc.vector.tensor_mul(out=eb16, in0=eb16, in1=rb16)
            nc.vector.tensor_add(out=eb16, in0=eb16, in1=mb16)
            b16r = cpool.tile([BS, OUT], F32R)
            nc.vector.tensor_add(out=b16r, in0=eb16, in1=psmu)

            # ---- main streaming loop ------------------------------------
            for b in range(BS):
                last = b == BS - 1
                ps = ppool.tile([1, OUT], F32)
                ep = epool.tile([P, FREE], F32, tag="ep")
                eps_src = eps_d[b, :, :].rearrange("(c p) o -> p c o", p=P)
                if not last:
                    nc.sync.dma_start(out=ep, in_=eps_src)
                else:
                    for c in range(CPP):
                        nc.sync.dma_start(
                            out=ep[:, c * OUT : (c + 1) * OUT],
                            in_=eps_src[:, c : c + 1, :],
                        )
                nq = 2 if not last else CPP
                cw = CPP // nq  # chunks per TT
                for q in range(nq):
                    epr = eprpool.tile([P, FREE // 2], F32R, tag="epr")
                    nc.vector.tensor_mul(
                        out=epr[:, : cw * OUT],
                        in0=ep[:, q * cw * OUT : (q + 1) * cw * OUT],
                        in1=sig[:, q * cw * OUT : (q + 1) * cw * OUT],
                    )
                    for c2 in range(cw):
                        c = cw * q + c2
                        col = xtr[:, c * BS + b : c * BS + b + 1]
                        for nh in range(2):
                            nc.tensor.matmul(
                                ps[0:1, nh * 512 : (nh + 1) * 512],
                                col,
                                epr[:, c2 * OUT + nh * 512 : c2 * OUT + (nh + 1) * 512],
                                start=(q == 0 and c2 == 0),
                                stop=False,
                            )
                # one-hot matmul adds bias row b into the partition-0 PSUM row
                for nh in range(2):
                    nc.tensor.matmul(
                        ps[0:1, nh * 512 : (nh + 1) * 512],
                        idr[:, b : b + 1],
                        b16r[:, nh * 512 : (nh + 1) * 512],
                        start=False,
                        stop=True,
                    )
                orow = spool.tile([1, OUT], F32)
                nc.scalar.copy(orow, ps[0:1, :])
                nc.scalar.dma_start(out=out_d[b : b + 1, :], in_=orow)

    nc.finalize()
    return nc


_NC_CACHE = None


def _get_nc():
    global _NC_CACHE
    if _NC_CACHE is None:
        _NC_CACHE = build_nc()
    return _NC_CACHE


def kernel(x, mu, ro, mu_bias, ro_bias, eps, eps_bias, _trace=False, _tmpdir=None):
    x = np.ascontiguousarray(np.asarray(x, dtype=np.float32))
    mu = np.ascontiguousarray(np.asarray(mu, dtype=np.float32))
    ro = np.ascontiguousarray(np.asarray(ro, dtype=np.float32))
    mu_bias = np.asarray(mu_bias, dtype=np.float32).reshape(1, OUT)
    ro_bias = np.asarray(ro_bias, dtype=np.float32).reshape(1, OUT)
    eps = np.asarray(eps, dtype=np.float32)
    eps_bias = np.ascontiguousarray(np.asarray(eps_bias, dtype=np.float32))

    nc = _get_nc()

    zeros_bs = np.zeros((BS, OUT), dtype=np.float32)
    rb_full = np.ascontiguousarray(np.broadcast_to(ro_bias, (BS, OUT)))
    mb_full = np.ascontiguousarray(np.broadcast_to(mu_bias, (BS, OUT)))

    in_maps = []
    for core in range(NCORES):
        g, j = core // ISH, core % ISH
        b0, b1 = g * BS, (g + 1) * BS
        i0, i1 = j * INS, (j + 1) * INS
        # xt[p, c*BS + b] = x[b, i0 + c*128 + p]
        xt = np.ascontiguousarray(
            x[b0:b1, i0:i1].reshape(BS, CPP, P).transpose(2, 1, 0).reshape(P, CPP * BS)
        )
        in_maps.append(
            {
                "eps": np.ascontiguousarray(eps[b0:b1, i0:i1, :]),
                "ro": np.ascontiguousarray(ro[i0:i1, :]),
                "mu": np.ascontiguousarray(mu[i0:i1, :]),
                "xt": xt,
                "eps_bias": eps_bias[b0:b1] if j == 0 else zeros_bs,
                "ro_bias": rb_full,
                "mu_bias": mb_full if j == 0 else zeros_bs,
            }
        )

    res = run_bass_kernel_spmd(
        nc, in_maps, core_ids=list(range(NCORES)), trace=_trace, tmpdir=_tmpdir
    )
    out = np.empty((B, OUT), dtype=np.float32)
    for g in range(BG):
        acc = res.results[g * ISH]["out"].copy()
        for j in range(1, ISH):
            acc += res.results[g * ISH + j]["out"]
        out[g * BS : (g + 1) * BS] = acc
    if _trace:
        kernel.last_results = res
    return out

